# revision 1
# baseline (speedup 1.0000x reference)
"""Routed (sparse) MoE kernel for Trainium2, expert-parallel over 8 NeuronCores.

Problem: Qwen3-MoE sparse block. T=2048 tokens, H=2048 hidden, E=32 experts,
F=768 intermediate, top-K=8, norm_topk_prob=True.

Strategy:
  * Host: router (logits -> softmax -> top-8 -> renormalize), replicated with
    jax-on-CPU to match the reference's numerics bit-for-bit where possible.
  * Host: gather each expert's routed tokens into a fixed-capacity (512) slot,
    pre-transposed to [H, C] and cast to bf16. Expert e -> core e%8, slot e//8.
    Tokens beyond capacity (rare: mean count is 512) are computed on host in
    fp32 — this keeps the device graph shape input-independent.
  * Device (per core): 4 expert slots. For each slot, the whole SwiGLU FFN in
    a transposed dataflow (tokens on the matmul free axis), bf16 matmuls with
    fp32 PSUM accumulation, silu on ACT, multiply on DVE:
        gT[F,C] = Wg^T x      (lhsT = Wg[H,F] tiles, rhs = xT[H,C] tiles)
        uT[F,C] = Wu^T x
        hT      = silu(gT) * uT
        yT[H,C] = Wd^T h      (lhsT = Wd[F,H] tiles, rhs = hT tiles)
    No on-chip transposes anywhere.
  * Host: combine — out[t] = sum_k w[t,k] * y_{e_k}[t], a per-expert weighted
    scatter-add with unique indices (fp32).
"""

import numpy as np
import ml_dtypes

import concourse.bass as bass  # noqa: F401  (registers engines)
import concourse.mybir as mybir
import concourse.tile as tile
from concourse import bacc
from concourse.bass_utils import run_bass_kernel_spmd

# Model dims (hardcoded per problem spec)
T, H, E, F, K = 2048, 2048, 32, 768, 8
NCORES = 8
SLOTS = E // NCORES  # 4 expert slots per core
C = 512              # per-expert token capacity on device
P = 128
KH = H // P          # 16 k-tiles over hidden
MF = F // P          # 6  m-tiles over intermediate
KF = F // P          # 6  k-tiles over intermediate (down proj)
MH = H // P          # 16 m-tiles over hidden (down proj)

BF16 = mybir.dt.bfloat16
F32 = mybir.dt.float32

# Exposed for test harnesses: the BassKernelResults of the last device run.
LAST_RESULT = None

_NC_CACHE = None


def _build_graph():
    """One SPMD graph, identical for all 8 cores (only input data differs)."""
    nc = bacc.Bacc("TRN2", target_bir_lowering=False, debug=False,
                   num_devices=NCORES)
    xt_d = nc.dram_tensor("xt", [SLOTS, H, C], BF16, kind="ExternalInput").ap()
    wg_d = nc.dram_tensor("wg", [SLOTS, H, F], BF16, kind="ExternalInput").ap()
    wu_d = nc.dram_tensor("wu", [SLOTS, H, F], BF16, kind="ExternalInput").ap()
    wd_d = nc.dram_tensor("wd", [SLOTS, F, H], BF16, kind="ExternalInput").ap()
    y_d = nc.dram_tensor("y", [SLOTS, H, C], BF16, kind="ExternalOutput").ap()

    DCH = 2   # k-tiles per wd load chunk
    GA = 4    # gate-ahead depth (psg tiles in flight)
    # x/wg/wu are loaded in 4-k-tile chunks: one DMA instruction each (the
    # ~640ns/instruction serial issue cost on the sync sequencer adds up)
    # while keeping per-partition contiguous runs small enough (~1-1.5KB)
    # that DMA SBUF writes don't starve the PE's operand streaming
    CHUNKS = [(0, 4), (4, 4), (8, 4), (12, 4)]
    K2CHUNK = {}
    for ci, (k0, nk) in enumerate(CHUNKS):
        for k in range(k0, k0 + nk):
            K2CHUNK[k] = (ci, k - k0)

    with tile.TileContext(nc) as tc:
        with (
            tc.tile_pool(name="warm", bufs=1) as warm,
            tc.tile_pool(name="xp", bufs=4) as xp,
            tc.tile_pool(name="wgp", bufs=4) as wgp,
            tc.tile_pool(name="wup", bufs=4) as wup,
            tc.tile_pool(name="wdp", bufs=4) as wdp,
            tc.tile_pool(name="hp", bufs=2 * MF) as hp,
            tc.tile_pool(name="sp", bufs=3) as sp,
            tc.tile_pool(name="yp", bufs=8) as yp,
            tc.tile_pool(name="psA", bufs=GA, space="PSUM") as psA,
            tc.tile_pool(name="ps", bufs=2, space="PSUM") as ps,
        ):
            # PE warm-up: trip the HAM activity window during the DMA lead-in
            wlhs = warm.tile([P, P], BF16, tag="wlhs")
            wrhs = warm.tile([P, C], BF16, tag="wrhs")
            nc.vector.memset(wlhs[:], 0.0)
            nc.vector.memset(wrhs[:], 0.0)
            wps = psA.tile([P, C], F32, tag="psg")
            for _ in range(12):
                nc.tensor.matmul(wps[:], wlhs[:], wrhs[:], start=True, stop=True)

            def preload_gate_up(s):
                x_t, wg_t, wu_t = [], [], []
                for ci, (k0, nk) in enumerate(CHUNKS):
                    xc = xp.tile([P, nk * C], BF16, tag=f"x{nk}", bufs=6)
                    nc.sync.dma_start(
                        xc[:].rearrange("p (ko c) -> p ko c", c=C),
                        xt_d[s, k0 * P:(k0 + nk) * P, :].rearrange(
                            "(ko p) c -> p ko c", p=P
                        ),
                    )
                    x_t.append(xc)
                    wc = wgp.tile([P, nk * F], BF16, tag=f"wg{nk}", bufs=8)
                    nc.sync.dma_start(
                        wc[:].rearrange("p (ko f) -> p ko f", f=F),
                        wg_d[s, k0 * P:(k0 + nk) * P, :].rearrange(
                            "(ko p) f -> p ko f", p=P
                        ),
                    )
                    wg_t.append(wc)
                for ci, (k0, nk) in enumerate(CHUNKS):
                    wc = wup.tile([P, nk * F], BF16, tag=f"wu{nk}", bufs=8)
                    nc.sync.dma_start(
                        wc[:].rearrange("p (ko f) -> p ko f", f=F),
                        wu_d[s, k0 * P:(k0 + nk) * P, :].rearrange(
                            "(ko p) f -> p ko f", p=P
                        ),
                    )
                    wu_t.append(wc)
                return x_t, wg_t, wu_t

            def preload_down(s):
                wd_t = []
                for c in range(KF // DCH):
                    wc = wdp.tile([P, DCH * H], BF16, tag="wd2", bufs=4)
                    nc.sync.dma_start(
                        wc[:].rearrange("p (ko h) -> p ko h", h=H),
                        wd_d[s, c * DCH * P:(c + 1) * DCH * P, :].rearrange(
                            "(ko p) h -> p ko h", p=P
                        ),
                    )
                    wd_t.append(wc)
                return wd_t

            def wslice(tiles, k, fdim, m):
                ci, off = K2CHUNK[k]
                return tiles[ci][:, off * fdim + m * P: off * fdim + (m + 1) * P]

            def xslice(tiles, k):
                ci, off = K2CHUNK[k]
                return tiles[ci][:, off * C:(off + 1) * C]

            nxt = preload_gate_up(0)
            for s in range(SLOTS):
                x_t, wg_t, wu_t = nxt
                wd_t = preload_down(s)

                def emit_gate(m):
                    psg = psA.tile([P, C], F32, tag="psg")
                    for k in range(KH):
                        nc.tensor.matmul(
                            psg[:], wslice(wg_t, k, F, m), xslice(x_t, k),
                            start=(k == 0), stop=(k == KH - 1),
                        )
                    return psg

                # software-pipelined gate-ahead
                psg_q = [emit_gate(m) for m in range(GA)]
                h_tiles = []
                for m in range(MF):
                    psu = ps.tile([P, C], F32, tag="psu")
                    for k in range(KH):
                        nc.tensor.matmul(
                            psu[:], wslice(wu_t, k, F, m), xslice(x_t, k),
                            start=(k == 0), stop=(k == KH - 1),
                        )
                    psg = psg_q[m % GA]
                    sil = sp.tile([P, C], F32, tag="sil")
                    nc.scalar.activation(
                        sil[:], psg[:], mybir.ActivationFunctionType.Silu
                    )
                    hm = hp.tile([P, C], BF16, tag="h")
                    nc.vector.tensor_tensor(
                        hm[:], sil[:], psu[:], mybir.AluOpType.mult
                    )
                    h_tiles.append(hm)
                    if m + GA < MF:
                        psg_q[(m + GA) % GA] = emit_gate(m + GA)
                    if m == 1 and s + 1 < SLOTS:
                        nxt = preload_gate_up(s + 1)

                # down projection; y stores issued from the scalar queue
                for mh in range(MH):
                    psy = ps.tile([P, C], F32, tag="psy")
                    for k in range(KF):
                        nc.tensor.matmul(
                            psy[:],
                            wd_t[k // DCH][:, (k % DCH) * H + mh * P:(k % DCH) * H + (mh + 1) * P],
                            h_tiles[k][:],
                            start=(k == 0), stop=(k == KF - 1),
                        )
                    yt = yp.tile([P, C], BF16, tag="y")
                    nc.vector.tensor_copy(out=yt[:], in_=psy[:])
                    nc.scalar.dma_start(y_d[s, mh * P:(mh + 1) * P, :], yt[:])

    nc.compile()
    return nc


def _route(x, gate_w):
    """Replicate the reference router. Returns (topk_idx, topk_w) as numpy."""
    try:
        import jax
        import jax.numpy as jnp

        cpu = jax.devices("cpu")[0]
        with jax.default_device(cpu):
            logits = jnp.asarray(x) @ jnp.asarray(gate_w)
            probs = jax.nn.softmax(logits.astype(jnp.float32), axis=-1)
            topk_w, topk_idx = jax.lax.top_k(probs, K)
            topk_w = topk_w / jnp.sum(topk_w, axis=-1, keepdims=True)
            return np.asarray(topk_idx), np.asarray(topk_w)
    except Exception:
        logits = x.astype(np.float32) @ gate_w.astype(np.float32)
        lm = logits.max(-1, keepdims=True)
        p = np.exp(logits - lm)
        p /= p.sum(-1, keepdims=True)
        topk_idx = np.argsort(-p, kind="stable", axis=-1)[:, :K]
        topk_w = np.take_along_axis(p, topk_idx, axis=-1)
        topk_w = topk_w / topk_w.sum(-1, keepdims=True)
        return topk_idx.astype(np.int32), topk_w


def _silu(v):
    return v / (1.0 + np.exp(-v))


def kernel(hidden_states, gate_w, w_gate_proj, w_up_proj, w_down_proj):
    global LAST_RESULT, _NC_CACHE

    x = np.asarray(hidden_states, dtype=np.float32)
    gate_w = np.asarray(gate_w, dtype=np.float32)
    wg_all = np.asarray(w_gate_proj, dtype=np.float32)
    wu_all = np.asarray(w_up_proj, dtype=np.float32)
    wd_all = np.asarray(w_down_proj, dtype=np.float32)

    # ---- Host router ----
    topk_idx, topk_w = _route(x, gate_w)

    # Per-expert token lists (kept on device up to capacity C; rest on host)
    route_w = np.zeros((T, E), np.float32)
    np.put_along_axis(route_w, topk_idx, topk_w.astype(np.float32), axis=-1)
    expert_tokens = [np.nonzero(route_w[:, e])[0] for e in range(E)]

    x_bf = x.astype(ml_dtypes.bfloat16)

    # ---- Build per-core inputs ----
    in_maps = []
    for core in range(NCORES):
        experts = [core + NCORES * s for s in range(SLOTS)]
        xt = np.zeros((SLOTS, H, C), ml_dtypes.bfloat16)
        for s, e in enumerate(experts):
            idx = expert_tokens[e][:C]
            xt[s, :, : len(idx)] = x_bf[idx].T
        in_maps.append(
            {
                "xt": xt,
                "wg": np.ascontiguousarray(wg_all[experts]).astype(ml_dtypes.bfloat16),
                "wu": np.ascontiguousarray(wu_all[experts]).astype(ml_dtypes.bfloat16),
                "wd": np.ascontiguousarray(wd_all[experts]).astype(ml_dtypes.bfloat16),
            }
        )

    # ---- Device run ----
    if _NC_CACHE is None:
        _NC_CACHE = _build_graph()
    nc = _NC_CACHE
    res = run_bass_kernel_spmd(nc, in_maps, core_ids=list(range(NCORES)))
    LAST_RESULT = res

    # ---- Host combine ----
    out = np.zeros((T, H), np.float32)
    for e in range(E):
        core, s = e % NCORES, e // NCORES
        idx = expert_tokens[e]
        kept, ov = idx[:C], idx[C:]
        yT = np.asarray(res.results[core]["y"][s]).astype(np.float32)  # [H, C]
        w_kept = route_w[kept, e]
        out[kept] += w_kept[:, None] * yT[:, : len(kept)].T
        if len(ov):
            xo = x[ov]
            h = _silu(xo @ wg_all[e]) * (xo @ wu_all[e])
            out[ov] += route_w[ov, e][:, None] * (h @ wd_all[e])

    return out

